# revision 2
# baseline (speedup 1.0000x reference)
"""Block-diagonal matmul (BlockLinear) on 8 Trainium2 NeuronCores.

Problem: W [16, 64, 64] f32 stacked square blocks; inp [1024, 32768] f32.
out = block_diag(W) @ inp, i.e. per-block out[h] = W[h] @ inp[h*64:(h+1)*64, :].

Strategy (data parallel over the batch axis, per the sharding hint):
  - Shard inp / out along B=32768 across 8 cores (4096 columns each).
  - Host-side, pack the 16 64x64 blocks into 8 block-diagonal 128x128 pairs,
    pre-transposed for the TensorE "lhsT" stationary operand (so the device
    does no transposes and the full 128-partition dim is used).
  - Per core: for each of the 8 row-pairs, DMA a [128, 4096] f32 slab in
    (2 MiB, HWDGE on the sync engine), run 8 matmuls of N=512 f32 into PSUM
    banks, copy PSUM->SBUF on VectorE, and DMA the result out in 2 chunks on
    the scalar-engine HWDGE ring (separate FIFO from loads; finer store
    granularity shortens the pipeline tail).

Memory-bound: 32.25 MiB HBM traffic per core ~= 94 us at ~358 GB/s per-core
HBM bandwidth. Measured (repeat-loop slope on HW): ~95 us per core.
"""

import os
import sys

import numpy as np

for _p in ("/opt/trn_rl_repo", "/opt/pypackages"):
    if os.path.isdir(_p) and _p not in sys.path:
        sys.path.append(_p)

H, D_BLK = 16, 64
D_TOTAL = H * D_BLK            # 1024
B = 32768
N_CORES = 8
BS = B // N_CORES              # 4096 batch columns per core
N_PAIR = H // 2                # 8 pairs of blocks -> 128 partitions each
FREE = 512                     # one PSUM bank of f32
NT = BS // FREE                # 8 matmuls per pair

_CACHE = {}


def _build_program(repeat: int = 1, variant: dict | None = None):
    import concourse.bacc as bacc
    import concourse.tile as tile
    from concourse import mybir

    # Defaults = best HW-measured variant (A/B at same For_i repeat R):
    # deep double-buffering, stores in 2x1MiB chunks on the scalar HWDGE
    # ring (separate FIFO from loads), last pair stored in 4 finer chunks,
    # PSUM->SBUF copies in 2-bank [128,1024] chunks split DVE(3)/ACT(1) to
    # shorten the per-pair copy chain, weight load off the sync ring.
    v = dict(bufs_x=4, bufs_y=4, store_chunks=2, load_chunks=1,
             alt_engines=False, copy_act_from=6, last_sc=4,
             w_on_scalar=True, load_merge=1, phased=False, copy_span=2,
             last_lc=None)
    v.update(variant or {})

    f32 = mybir.dt.float32
    nc = bacc.Bacc("TRN2", target_bir_lowering=False, debug=False,
                   num_devices=N_CORES)

    w_d = nc.dram_tensor("w", (128, N_PAIR * 128), f32, kind="ExternalInput")
    x_d = nc.dram_tensor("x", (N_PAIR, 128, BS), f32, kind="ExternalInput")
    y_d = nc.dram_tensor("y", (N_PAIR, 128, BS), f32, kind="ExternalOutput")

    with tile.TileContext(nc) as tc:
        with (
            tc.tile_pool(name="wpool", bufs=1) as wpool,
            tc.tile_pool(name="xpool", bufs=v["bufs_x"]) as xpool,
            tc.tile_pool(name="ypool", bufs=v["bufs_y"]) as ypool,
            tc.tile_pool(name="psum", bufs=8 // v["copy_span"],
                         space="PSUM") as psum_pool,
        ):
            wt = wpool.tile([128, N_PAIR * 128], f32)
            (nc.scalar if v["w_on_scalar"] else nc.sync).dma_start(wt[:], w_d[:])

            x_r = x_d.rearrange("p k b -> k p b")
            y_r = y_d.rearrange("p k b -> k p b")

            def phased_body():
                # Pure-read phase (all x loads), then pure-write phase
                # (stores gated on the last load) -- avoids HBM read/write
                # bus turnaround at packet granularity.
                from concourse.tile_rust import add_dep_helper

                sc = v["store_chunks"]
                xts = []
                last_ld = None
                for p in range(N_PAIR):
                    xt = xpool.tile([128, 1, BS], f32)
                    last_ld = nc.sync.dma_start(xt[:, :, :], x_r[:, p:p + 1, :])
                    xts.append(xt)
                for p in range(N_PAIR):
                    yt = ypool.tile([128, 1, BS], f32)
                    for n in range(NT):
                        ps = psum_pool.tile([128, FREE], f32)
                        nc.tensor.matmul(
                            ps[:],
                            wt[:, p * 128:(p + 1) * 128],
                            xts[p][:, 0, n * FREE:(n + 1) * FREE],
                            start=True, stop=True,
                        )
                        if n >= v["copy_act_from"]:
                            nc.scalar.copy(yt[:, 0, n * FREE:(n + 1) * FREE],
                                           ps[:])
                        else:
                            nc.vector.tensor_copy(
                                yt[:, 0, n * FREE:(n + 1) * FREE], ps[:])
                    for i in range(sc):
                        w_ = BS // sc
                        st = nc.scalar.dma_start(
                            y_r[:, p:p + 1, i * w_:(i + 1) * w_],
                            yt[:, :, i * w_:(i + 1) * w_])
                        if p == 0 and i == 0:
                            add_dep_helper(
                                st.ins, last_ld.ins, sync=True,
                                reason="phase: stores begin after all loads")

            def body():
                lc, lm = v["load_chunks"], v["load_merge"]
                for pg in range(N_PAIR // lm):
                    sc = v["store_chunks"]
                    my_lc = lc
                    if pg == N_PAIR // lm - 1:
                        if v["last_sc"]:
                            sc = v["last_sc"]
                        if v["last_lc"]:
                            my_lc = v["last_lc"]
                    if v["alt_engines"] and pg % 2:
                        ld_eng, st_eng = nc.scalar, nc.sync
                    else:
                        ld_eng, st_eng = nc.sync, nc.scalar
                    # xt holds lm pairs: [128, lm, BS]
                    xt = xpool.tile([128, lm, BS], f32)
                    for i in range(my_lc):
                        w_ = BS // my_lc
                        ld_eng.dma_start(
                            xt[:, :, i * w_:(i + 1) * w_],
                            x_r[:, pg * lm:(pg + 1) * lm, i * w_:(i + 1) * w_])
                    yt = ypool.tile([128, lm, BS], f32)
                    span = v["copy_span"]
                    for j in range(lm):
                        p = pg * lm + j
                        for n2 in range(NT // span):
                            ps = psum_pool.tile([128, span * FREE], f32)
                            for s in range(span):
                                n = n2 * span + s
                                nc.tensor.matmul(
                                    ps[:, s * FREE:(s + 1) * FREE],
                                    wt[:, p * 128:(p + 1) * 128],
                                    xt[:, j, n * FREE:(n + 1) * FREE],
                                    start=True, stop=True,
                                )
                            lo = n2 * span * FREE
                            hi = lo + span * FREE
                            if n2 * span >= v["copy_act_from"]:
                                nc.scalar.copy(yt[:, j, lo:hi], ps[:])
                            else:
                                nc.vector.tensor_copy(yt[:, j, lo:hi], ps[:])
                    for i in range(sc * lm):
                        w_ = BS // sc
                        j, ii = divmod(i, sc)
                        st_eng.dma_start(
                            y_r[:, pg * lm + j, ii * w_:(ii + 1) * w_],
                            yt[:, j, ii * w_:(ii + 1) * w_])

            the_body = phased_body if v["phased"] else body
            if repeat == 1:
                the_body()
            else:
                with tc.For_i(0, repeat, 1):
                    the_body()

    nc.compile()
    return nc


def _get_program(repeat: int = 1, variant: dict | None = None):
    key = ("nc", repeat, tuple(sorted((variant or {}).items())))
    if key not in _CACHE:
        _CACHE[key] = _build_program(repeat, variant)
    return _CACHE[key]


def _pack_weights(W: np.ndarray) -> np.ndarray:
    """[16, 64, 64] -> [128, 8*128] lhsT layout: col p*128+m, row k holds
    block_diag(W[2p].T, W[2p+1].T)[k, m]."""
    WD = np.zeros((N_PAIR, 128, 128), dtype=np.float32)
    for p in range(N_PAIR):
        WD[p, :D_BLK, :D_BLK] = W[2 * p].T
        WD[p, D_BLK:, D_BLK:] = W[2 * p + 1].T
    return np.ascontiguousarray(WD.transpose(1, 0, 2).reshape(128, N_PAIR * 128))


def _get_runner():
    """Build (once) the jitted 8-core dispatch for the bass program.

    Mirrors concourse.bass2jax.run_bass_via_pjrt's multi-core branch, but is
    cached so repeat kernel() calls skip retracing, and takes pre-concatenated
    global inputs to avoid an extra host copy.
    """
    if "runner" in _CACHE:
        return _CACHE["runner"]

    import jax
    from concourse import mybir
    from concourse.bass2jax import (
        _bass_exec_p,
        install_neuronx_cc_hook,
        partition_id_tensor,
    )
    from jax.experimental.shard_map import shard_map
    from jax.sharding import Mesh, NamedSharding, PartitionSpec

    install_neuronx_cc_hook()
    nc = _get_program()

    partition_name = nc.partition_id_tensor.name if nc.partition_id_tensor else None
    in_names, out_names, out_avals, out_shapes = [], [], [], []
    for alloc in nc.m.functions[0].allocations:
        if not isinstance(alloc, mybir.MemoryLocationSet):
            continue
        name = alloc.memorylocations[0].name
        if alloc.kind == "ExternalInput":
            if name != partition_name:
                in_names.append(name)
        elif alloc.kind == "ExternalOutput":
            out_names.append(name)
            shape = tuple(alloc.tensor_shape)
            dtype = mybir.dt.np(alloc.dtype)
            out_avals.append(jax.core.ShapedArray(shape, dtype))
            out_shapes.append((shape, dtype))
    n_params = len(in_names)
    n_outs = len(out_avals)
    all_in_names = in_names + out_names
    if partition_name is not None:
        all_in_names.append(partition_name)
    donate = tuple(range(n_params, n_params + n_outs))

    def _body(*args):
        operands = list(args)
        if partition_name is not None:
            operands.append(partition_id_tensor())
        outs = _bass_exec_p.bind(
            *operands,
            out_avals=tuple(out_avals),
            in_names=tuple(all_in_names),
            out_names=tuple(out_names),
            lowering_input_output_aliases=(),
            sim_require_finite=True,
            sim_require_nnan=True,
            nc=nc,
        )
        return tuple(outs)

    devices = jax.devices()[:N_CORES]
    mesh = Mesh(np.asarray(devices), ("core",))
    in_specs = (PartitionSpec("core"),) * (n_params + n_outs)
    out_specs = (PartitionSpec("core"),) * n_outs
    sharded = jax.jit(
        shard_map(_body, mesh=mesh, in_specs=in_specs, out_specs=out_specs,
                  check_rep=False),
        donate_argnums=donate,
        keep_unused=True,
    )
    shard = NamedSharding(mesh, PartitionSpec("core"))

    # Donated output buffers: create on-device (no 128 MB host->device
    # transfer per call). Fall back to host staging if the backend rejects
    # the output-only jit.
    import jax.numpy as jnp

    zero_shapes = [((shape[0] * N_CORES,) + shape[1:], dtype)
                   for shape, dtype in out_shapes]
    zeros_jit = jax.jit(
        lambda: tuple(jnp.zeros(s, d) for s, d in zero_shapes),
        out_shardings=tuple(shard for _ in zero_shapes),
    )

    def host_zeros():
        return [jax.device_put(np.zeros(s, d), shard) for s, d in zero_shapes]

    try:
        jax.block_until_ready(zeros_jit())
        make_zeros = lambda: list(zeros_jit())  # noqa: E731
    except Exception:
        make_zeros = host_zeros

    def run(global_ins: dict):
        """global_ins: name -> concatenated [N_CORES*dim0, ...] array."""
        dev_in = [jax.device_put(global_ins[name], shard)
                  for name in in_names]
        outs = sharded(*dev_in, *make_zeros())
        return {name: np.asarray(o) for name, o in zip(out_names, outs)}

    _CACHE["runner"] = run
    return run


def _make_global_inputs(W: np.ndarray, inp: np.ndarray) -> dict:
    # Global sharded inputs (axis 0 split across cores by shard_map):
    #   w: [N_CORES*128, 1024] -- weights replicated per core
    #   x: [N_CORES*8, 128, BS] -- core c gets inp[:, c*BS:(c+1)*BS]
    w_host = _pack_weights(np.asarray(W, dtype=np.float32))
    w_global = np.tile(w_host, (N_CORES, 1))
    x_global = np.ascontiguousarray(
        np.asarray(inp, dtype=np.float32)
        .reshape(N_PAIR, 128, N_CORES, BS).transpose(2, 0, 1, 3)
    ).reshape(N_CORES * N_PAIR, 128, BS)
    return {"w": w_global, "x": x_global}


def _kernel_direct(w_host: np.ndarray, inp: np.ndarray) -> np.ndarray:
    w_global = np.tile(w_host, (N_CORES, 1))
    x_global = np.ascontiguousarray(
        inp.reshape(N_PAIR, 128, N_CORES, BS).transpose(2, 0, 1, 3)
    ).reshape(N_CORES * N_PAIR, 128, BS)

    run = _get_runner()
    outs = run({"w": w_global, "x": x_global})

    y = outs["y"].reshape(N_CORES, N_PAIR, 128, BS)
    return np.ascontiguousarray(y.transpose(1, 2, 0, 3)).reshape(D_TOTAL, B)


def _kernel_via_spmd(w_host: np.ndarray, inp: np.ndarray) -> np.ndarray:
    from concourse.bass_utils import run_bass_kernel_spmd

    nc = _get_program()
    in_maps = []
    for c in range(N_CORES):
        x_shard = np.ascontiguousarray(inp[:, c * BS:(c + 1) * BS])
        in_maps.append({"w": w_host, "x": x_shard.reshape(N_PAIR, 128, BS)})
    res = run_bass_kernel_spmd(nc, in_maps, core_ids=list(range(N_CORES)))
    out = np.empty((D_TOTAL, B), dtype=np.float32)
    for c in range(N_CORES):
        out[:, c * BS:(c + 1) * BS] = res.results[c]["y"].reshape(D_TOTAL, BS)
    return out


def kernel(W: np.ndarray, inp: np.ndarray) -> np.ndarray:
    W = np.asarray(W, dtype=np.float32)
    inp = np.asarray(inp, dtype=np.float32)
    assert W.shape == (H, D_BLK, D_BLK) and inp.shape == (D_TOTAL, B)

    w_host = _pack_weights(W)

    try:
        from concourse._compat import axon_active
        use_direct = axon_active()
    except Exception:
        use_direct = False

    if use_direct:
        try:
            return _kernel_direct(w_host, inp)
        except Exception:
            # Transient device wedges (NRT_EXEC_UNIT_UNRECOVERABLE) have been
            # observed to need ~60 s to clear; retry once after a long
            # backoff, then fall back to the run_bass_kernel_spmd path.
            import time
            time.sleep(45)
            try:
                return _kernel_direct(w_host, inp)
            except Exception:
                time.sleep(30)
    return _kernel_via_spmd(w_host, inp)


if __name__ == "__main__":
    rng = np.random.default_rng(0)
    W = rng.standard_normal((H, D_BLK, D_BLK), dtype=np.float32)
    inp = rng.standard_normal((D_TOTAL, B), dtype=np.float32)
    out = kernel(W, inp)
    ref = np.einsum("hij,hjb->hib", W, inp.reshape(H, D_BLK, B)).reshape(D_TOTAL, B)
    err = np.abs(out - ref).max() / max(np.abs(ref).max(), 1e-9)
    print("self-check rel err:", err)



# revision 13
# speedup vs baseline: 1.9047x; 1.9047x over previous
"""Block-diagonal matmul (BlockLinear) on 8 Trainium2 NeuronCores.

Problem: W [16, 64, 64] f32 stacked square blocks; inp [1024, 32768] f32.
out = block_diag(W) @ inp, i.e. per-block out[h] = W[h] @ inp[h*64:(h+1)*64, :].

Strategy (data parallel over the batch axis, per the sharding hint):
  - Shard inp / out along B=32768 across 8 cores (4096 columns each).
  - Host-side, pack the 16 64x64 blocks into 8 block-diagonal 128x128 pairs,
    pre-transposed for the TensorE "lhsT" stationary operand (so the device
    does no transposes and the full 128-partition dim is used), and cast all
    HBM-facing data to fp16 (tolerance allows it: fp16 matmul with f32 PSUM
    accumulation keeps rel err ~1e-3, and it halves HBM traffic, which is
    the binding constraint).
  - Per core: for each of the 8 row-pairs, DMA a [128, 4096] fp16 slab in
    (1 MiB, HWDGE on the sync engine), run 8 matmuls of N=512 into f32 PSUM
    banks, copy PSUM->SBUF with fused f32->fp16 conversion on VectorE/ScalarE,
    and DMA the result out on the scalar-engine HWDGE ring (separate FIFO
    from loads; finer store granularity shortens the pipeline tail).
  - Host upconverts the fp16 device output back to f32.

Memory-bound: 16.25 MiB HBM traffic per core ~= 47.5 us at ~358 GB/s
per-core HBM bandwidth.
"""

import os
import sys

import numpy as np

for _p in ("/opt/trn_rl_repo", "/opt/pypackages"):
    if os.path.isdir(_p) and _p not in sys.path:
        sys.path.append(_p)

H, D_BLK = 16, 64
D_TOTAL = H * D_BLK            # 1024
B = 32768
N_CORES = 8
BS = B // N_CORES              # 4096 batch columns per core
N_PAIR = H // 2                # 8 pairs of blocks -> 128 partitions each
FREE = 512                     # one PSUM bank of f32
NT = BS // FREE                # 8 matmuls per pair

_CACHE = {}


def _build_program(repeat: int = 1, variant: dict | None = None):
    import concourse.bacc as bacc
    import concourse.tile as tile
    from concourse import mybir

    # Defaults = best HW-measured variant (A/B at same For_i repeat R):
    # deep double-buffering, stores in 2 chunks on the scalar HWDGE
    # ring (separate FIFO from loads), last pair stored in 4 finer chunks,
    # PSUM->SBUF copies in 2-bank [128,1024] chunks split DVE(3)/ACT(1) to
    # shorten the per-pair copy chain, weight load off the sync ring.
    # io=fp16: HBM-facing tensors in float16 (matmul in fp16, PSUM f32,
    # conversion fused into the PSUM->SBUF copies) -- halves HBM traffic.
    v = dict(bufs_x=4, bufs_y=4, store_chunks=2, load_chunks=1,
             alt_engines=False, copy_act_from=6, last_sc=4,
             w_on_scalar=True, load_merge=1, phased=False, copy_span=2,
             last_lc=None, io="fp16")
    v.update(variant or {})

    f32 = mybir.dt.float32
    tio = {"fp16": mybir.dt.float16, "bf16": mybir.dt.bfloat16,
           "f32": mybir.dt.float32}[v["io"]]
    nc = bacc.Bacc("TRN2", target_bir_lowering=False, debug=False,
                   num_devices=N_CORES)

    w_d = nc.dram_tensor("w", (128, N_PAIR * 128), tio, kind="ExternalInput")
    x_d = nc.dram_tensor("x", (N_PAIR, 128, BS), tio, kind="ExternalInput")
    y_d = nc.dram_tensor("y", (N_PAIR, 128, BS), tio, kind="ExternalOutput")

    with tile.TileContext(nc) as tc:
        with (
            tc.tile_pool(name="wpool", bufs=1) as wpool,
            tc.tile_pool(name="xpool", bufs=v["bufs_x"]) as xpool,
            tc.tile_pool(name="ypool", bufs=v["bufs_y"]) as ypool,
            tc.tile_pool(name="psum", bufs=8 // v["copy_span"],
                         space="PSUM") as psum_pool,
        ):
            wt = wpool.tile([128, N_PAIR * 128], tio)
            (nc.scalar if v["w_on_scalar"] else nc.sync).dma_start(wt[:], w_d[:])

            x_r = x_d.rearrange("p k b -> k p b")
            y_r = y_d.rearrange("p k b -> k p b")

            def phased_body():
                # Pure-read phase (all x loads), then pure-write phase
                # (stores gated on the last load) -- avoids HBM read/write
                # bus turnaround at packet granularity.
                from concourse.tile_rust import add_dep_helper

                sc = v["store_chunks"]
                xts = []
                last_ld = None
                for p in range(N_PAIR):
                    xt = xpool.tile([128, 1, BS], tio)
                    last_ld = nc.sync.dma_start(xt[:, :, :], x_r[:, p:p + 1, :])
                    xts.append(xt)
                for p in range(N_PAIR):
                    yt = ypool.tile([128, 1, BS], tio)
                    for n in range(NT):
                        ps = psum_pool.tile([128, FREE], f32)
                        nc.tensor.matmul(
                            ps[:],
                            wt[:, p * 128:(p + 1) * 128],
                            xts[p][:, 0, n * FREE:(n + 1) * FREE],
                            start=True, stop=True,
                        )
                        if n >= v["copy_act_from"]:
                            nc.scalar.copy(yt[:, 0, n * FREE:(n + 1) * FREE],
                                           ps[:])
                        else:
                            nc.vector.tensor_copy(
                                yt[:, 0, n * FREE:(n + 1) * FREE], ps[:])
                    for i in range(sc):
                        w_ = BS // sc
                        st = nc.scalar.dma_start(
                            y_r[:, p:p + 1, i * w_:(i + 1) * w_],
                            yt[:, :, i * w_:(i + 1) * w_])
                        if p == 0 and i == 0:
                            add_dep_helper(
                                st.ins, last_ld.ins, sync=True,
                                reason="phase: stores begin after all loads")

            def body():
                lc, lm = v["load_chunks"], v["load_merge"]
                for pg in range(N_PAIR // lm):
                    sc = v["store_chunks"]
                    my_lc = lc
                    if pg == N_PAIR // lm - 1:
                        if v["last_sc"]:
                            sc = v["last_sc"]
                        if v["last_lc"]:
                            my_lc = v["last_lc"]
                    if v["alt_engines"] and pg % 2:
                        ld_eng, st_eng = nc.scalar, nc.sync
                    else:
                        ld_eng, st_eng = nc.sync, nc.scalar
                    # xt holds lm pairs: [128, lm, BS]
                    xt = xpool.tile([128, lm, BS], tio)
                    for i in range(my_lc):
                        w_ = BS // my_lc
                        ld_eng.dma_start(
                            xt[:, :, i * w_:(i + 1) * w_],
                            x_r[:, pg * lm:(pg + 1) * lm, i * w_:(i + 1) * w_])
                    yt = ypool.tile([128, lm, BS], tio)
                    span = v["copy_span"]
                    for j in range(lm):
                        p = pg * lm + j
                        for n2 in range(NT // span):
                            ps = psum_pool.tile([128, span * FREE], f32)
                            for s in range(span):
                                n = n2 * span + s
                                nc.tensor.matmul(
                                    ps[:, s * FREE:(s + 1) * FREE],
                                    wt[:, p * 128:(p + 1) * 128],
                                    xt[:, j, n * FREE:(n + 1) * FREE],
                                    start=True, stop=True,
                                )
                            lo = n2 * span * FREE
                            hi = lo + span * FREE
                            if n2 * span >= v["copy_act_from"]:
                                nc.scalar.copy(yt[:, j, lo:hi], ps[:])
                            else:
                                nc.vector.tensor_copy(yt[:, j, lo:hi], ps[:])
                    for i in range(sc * lm):
                        w_ = BS // sc
                        j, ii = divmod(i, sc)
                        st_eng.dma_start(
                            y_r[:, pg * lm + j, ii * w_:(ii + 1) * w_],
                            yt[:, j, ii * w_:(ii + 1) * w_])

            the_body = phased_body if v["phased"] else body
            if repeat == 1:
                the_body()
            else:
                with tc.For_i(0, repeat, 1):
                    the_body()

    nc.compile()
    return nc


def _get_program(repeat: int = 1, variant: dict | None = None):
    key = ("nc", repeat, tuple(sorted((variant or {}).items())))
    if key not in _CACHE:
        _CACHE[key] = _build_program(repeat, variant)
    return _CACHE[key]


IO_DTYPE = np.float16


def _pack_weights(W: np.ndarray) -> np.ndarray:
    """[16, 64, 64] -> [128, 8*128] lhsT layout: col p*128+m, row k holds
    block_diag(W[2p].T, W[2p+1].T)[k, m]."""
    WD = np.zeros((N_PAIR, 128, 128), dtype=np.float32)
    for p in range(N_PAIR):
        WD[p, :D_BLK, :D_BLK] = W[2 * p].T
        WD[p, D_BLK:, D_BLK:] = W[2 * p + 1].T
    return np.ascontiguousarray(
        WD.transpose(1, 0, 2).reshape(128, N_PAIR * 128).astype(IO_DTYPE))


def _get_runner():
    """Build (once) the jitted 8-core dispatch for the bass program.

    Mirrors concourse.bass2jax.run_bass_via_pjrt's multi-core branch, but is
    cached so repeat kernel() calls skip retracing, and takes pre-concatenated
    global inputs to avoid an extra host copy.
    """
    if "runner" in _CACHE:
        return _CACHE["runner"]

    import jax
    from concourse import mybir
    from concourse.bass2jax import (
        _bass_exec_p,
        install_neuronx_cc_hook,
        partition_id_tensor,
    )
    from jax.experimental.shard_map import shard_map
    from jax.sharding import Mesh, NamedSharding, PartitionSpec

    install_neuronx_cc_hook()
    nc = _get_program()

    partition_name = nc.partition_id_tensor.name if nc.partition_id_tensor else None
    in_names, out_names, out_avals, out_shapes = [], [], [], []
    for alloc in nc.m.functions[0].allocations:
        if not isinstance(alloc, mybir.MemoryLocationSet):
            continue
        name = alloc.memorylocations[0].name
        if alloc.kind == "ExternalInput":
            if name != partition_name:
                in_names.append(name)
        elif alloc.kind == "ExternalOutput":
            out_names.append(name)
            shape = tuple(alloc.tensor_shape)
            dtype = mybir.dt.np(alloc.dtype)
            out_avals.append(jax.core.ShapedArray(shape, dtype))
            out_shapes.append((shape, dtype))
    n_params = len(in_names)
    n_outs = len(out_avals)
    all_in_names = in_names + out_names
    if partition_name is not None:
        all_in_names.append(partition_name)
    donate = tuple(range(n_params, n_params + n_outs))

    def _body(*args):
        operands = list(args)
        if partition_name is not None:
            operands.append(partition_id_tensor())
        outs = _bass_exec_p.bind(
            *operands,
            out_avals=tuple(out_avals),
            in_names=tuple(all_in_names),
            out_names=tuple(out_names),
            lowering_input_output_aliases=(),
            sim_require_finite=True,
            sim_require_nnan=True,
            nc=nc,
        )
        return tuple(outs)

    devices = jax.devices()[:N_CORES]
    mesh = Mesh(np.asarray(devices), ("core",))
    in_specs = (PartitionSpec("core"),) * (n_params + n_outs)
    out_specs = (PartitionSpec("core"),) * n_outs
    sharded = jax.jit(
        shard_map(_body, mesh=mesh, in_specs=in_specs, out_specs=out_specs,
                  check_rep=False),
        donate_argnums=donate,
        keep_unused=True,
    )
    shard = NamedSharding(mesh, PartitionSpec("core"))

    # Donated output buffers: create on-device (no 128 MB host->device
    # transfer per call). Fall back to host staging if the backend rejects
    # the output-only jit.
    import jax.numpy as jnp

    zero_shapes = [((shape[0] * N_CORES,) + shape[1:], dtype)
                   for shape, dtype in out_shapes]
    zeros_jit = jax.jit(
        lambda: tuple(jnp.zeros(s, d) for s, d in zero_shapes),
        out_shardings=tuple(shard for _ in zero_shapes),
    )

    def host_zeros():
        return [jax.device_put(np.zeros(s, d), shard) for s, d in zero_shapes]

    try:
        jax.block_until_ready(zeros_jit())
        make_zeros = lambda: list(zeros_jit())  # noqa: E731
    except Exception:
        make_zeros = host_zeros

    def run(global_ins: dict):
        """global_ins: name -> concatenated [N_CORES*dim0, ...] array."""
        dev_in = [jax.device_put(global_ins[name], shard)
                  for name in in_names]
        outs = sharded(*dev_in, *make_zeros())
        return {name: np.asarray(o) for name, o in zip(out_names, outs)}

    _CACHE["runner"] = run
    return run


def _make_global_inputs(W: np.ndarray, inp: np.ndarray) -> dict:
    # Global sharded inputs (axis 0 split across cores by shard_map):
    #   w: [N_CORES*128, 1024] -- weights replicated per core
    #   x: [N_CORES*8, 128, BS] -- core c gets inp[:, c*BS:(c+1)*BS]
    w_host = _pack_weights(np.asarray(W, dtype=np.float32))
    w_global = np.tile(w_host, (N_CORES, 1))
    x_global = np.ascontiguousarray(
        np.asarray(inp, dtype=np.float32)
        .reshape(N_PAIR, 128, N_CORES, BS).transpose(2, 0, 1, 3)
        .astype(IO_DTYPE)
    ).reshape(N_CORES * N_PAIR, 128, BS)
    return {"w": w_global, "x": x_global}


def _kernel_direct(w_host: np.ndarray, inp: np.ndarray) -> np.ndarray:
    w_global = np.tile(w_host, (N_CORES, 1))
    x_global = np.ascontiguousarray(
        inp.reshape(N_PAIR, 128, N_CORES, BS).transpose(2, 0, 1, 3)
        .astype(IO_DTYPE)
    ).reshape(N_CORES * N_PAIR, 128, BS)

    run = _get_runner()
    outs = run({"w": w_global, "x": x_global})

    y = outs["y"].reshape(N_CORES, N_PAIR, 128, BS)
    return np.ascontiguousarray(
        y.transpose(1, 2, 0, 3).astype(np.float32)).reshape(D_TOTAL, B)


def _kernel_via_spmd(w_host: np.ndarray, inp: np.ndarray) -> np.ndarray:
    from concourse.bass_utils import run_bass_kernel_spmd

    nc = _get_program()
    in_maps = []
    for c in range(N_CORES):
        x_shard = np.ascontiguousarray(
            inp[:, c * BS:(c + 1) * BS].astype(IO_DTYPE))
        in_maps.append({"w": w_host, "x": x_shard.reshape(N_PAIR, 128, BS)})
    res = run_bass_kernel_spmd(nc, in_maps, core_ids=list(range(N_CORES)))
    out = np.empty((D_TOTAL, B), dtype=np.float32)
    for c in range(N_CORES):
        out[:, c * BS:(c + 1) * BS] = (
            res.results[c]["y"].reshape(D_TOTAL, BS).astype(np.float32))
    return out


def kernel(W: np.ndarray, inp: np.ndarray) -> np.ndarray:
    W = np.asarray(W, dtype=np.float32)
    inp = np.asarray(inp, dtype=np.float32)
    assert W.shape == (H, D_BLK, D_BLK) and inp.shape == (D_TOTAL, B)

    w_host = _pack_weights(W)

    try:
        from concourse._compat import axon_active
        use_direct = axon_active()
    except Exception:
        use_direct = False

    if use_direct:
        try:
            return _kernel_direct(w_host, inp)
        except Exception:
            # Transient device wedges (NRT_EXEC_UNIT_UNRECOVERABLE) have been
            # observed to need ~60 s to clear; retry once after a long
            # backoff, then fall back to the run_bass_kernel_spmd path.
            import time
            time.sleep(45)
            try:
                return _kernel_direct(w_host, inp)
            except Exception:
                time.sleep(30)
    return _kernel_via_spmd(w_host, inp)


if __name__ == "__main__":
    rng = np.random.default_rng(0)
    W = rng.standard_normal((H, D_BLK, D_BLK), dtype=np.float32)
    inp = rng.standard_normal((D_TOTAL, B), dtype=np.float32)
    out = kernel(W, inp)
    ref = np.einsum("hij,hjb->hib", W, inp.reshape(H, D_BLK, B)).reshape(D_TOTAL, B)
    err = np.abs(out - ref).max() / max(np.abs(ref).max(), 1e-9)
    print("self-check rel err:", err)
    assert err < 2e-2, f"FAIL rel={err}"



# revision 23
# speedup vs baseline: 2.0038x; 1.0520x over previous
"""Block-diagonal matmul (BlockLinear) on 8 Trainium2 NeuronCores.

Problem: W [16, 64, 64] f32 stacked square blocks; inp [1024, 32768] f32.
out = block_diag(W) @ inp, i.e. per-block out[h] = W[h] @ inp[h*64:(h+1)*64, :].

Strategy (data parallel over the batch axis, per the sharding hint):
  - Shard inp / out along B=32768 across 8 cores (4096 columns each).
  - Host-side, pack the 16 64x64 blocks into 8 block-diagonal 128x128 pairs,
    pre-transposed for the TensorE "lhsT" stationary operand (so the device
    does no transposes and the full 128-partition dim is used), and cast all
    HBM-facing data to fp16 (tolerance allows it: fp16 matmul with f32 PSUM
    accumulation keeps rel err ~1e-3, and it halves HBM traffic, which is
    the binding constraint).
  - Per core: for each of the 8 row-pairs, DMA a [128, 4096] fp16 slab in
    (1 MiB, HWDGE on the sync engine), run 8 matmuls of N=512 into f32 PSUM
    banks, copy PSUM->SBUF with fused f32->fp16 conversion on VectorE/ScalarE,
    and DMA the result out on the scalar-engine HWDGE ring (separate FIFO
    from loads; finer store granularity shortens the pipeline tail).
  - Host upconverts the fp16 device output back to f32.

Memory-bound: 16.25 MiB HBM traffic per core ~= 47.5 us at ~358 GB/s
per-core HBM bandwidth.
"""

import os
import sys

import numpy as np

for _p in ("/opt/trn_rl_repo", "/opt/pypackages"):
    if os.path.isdir(_p) and _p not in sys.path:
        sys.path.append(_p)

H, D_BLK = 16, 64
D_TOTAL = H * D_BLK            # 1024
B = 32768
N_CORES = 8
BS = B // N_CORES              # 4096 batch columns per core
N_PAIR = H // 2                # 8 pairs of blocks -> 128 partitions each
FREE = 512                     # one PSUM bank of f32
NT = BS // FREE                # 8 matmuls per pair

_CACHE = {}


def _build_program(repeat: int = 1, variant: dict | None = None):
    import concourse.bacc as bacc
    import concourse.tile as tile
    from concourse import mybir

    # Defaults = best HW-measured variant (A/B at same For_i repeat R):
    # deep double-buffering, stores in 2 chunks on the scalar HWDGE
    # ring (separate FIFO from loads), last pair stored in 4 finer chunks,
    # PSUM->SBUF copies in 2-bank [128,1024] chunks split DVE(3)/ACT(1) to
    # shorten the per-pair copy chain, weight load off the sync ring.
    # io=fp16: HBM-facing tensors in float16 (matmul in fp16, PSUM f32,
    # conversion fused into the PSUM->SBUF copies) -- halves HBM traffic.
    # Best HW-measured config: fp16 IO; per iteration both HWDGE rings
    # first carry half the loads each (read-pure phase), then half the
    # stores each (write-pure phase) -- one read<->write direction switch
    # per ring per iteration, which matters because per-core HBM bandwidth
    # is shared between directions and mixing costs ~8%. Single whole-pair
    # store DMAs, deep pools, 4x unrolled hardware loop.
    v = dict(bufs_x=8, bufs_y=6, store_chunks=1, load_chunks=1,
             alt_engines=False, copy_act_from=6, last_sc=None,
             w_on_scalar=True, load_merge=1, phased=False, copy_span=2,
             last_lc=None, io="fp16", probe=None, ring="halves",
             phase_loads=True, store_merge=1, unroll=4)
    v.update(variant or {})

    f32 = mybir.dt.float32
    tio = {"fp16": mybir.dt.float16, "bf16": mybir.dt.bfloat16,
           "f32": mybir.dt.float32}[v["io"]]
    nc = bacc.Bacc("TRN2", target_bir_lowering=False, debug=False,
                   num_devices=N_CORES)

    w_d = nc.dram_tensor("w", (128, N_PAIR * 128), tio, kind="ExternalInput")
    x_d = nc.dram_tensor("x", (N_PAIR, 128, BS), tio, kind="ExternalInput")
    y_d = nc.dram_tensor("y", (N_PAIR, 128, BS), tio, kind="ExternalOutput")

    with tile.TileContext(nc) as tc:
        with (
            tc.tile_pool(name="wpool", bufs=1) as wpool,
            tc.tile_pool(name="xpool", bufs=v["bufs_x"]) as xpool,
            tc.tile_pool(name="ypool", bufs=v["bufs_y"]) as ypool,
            tc.tile_pool(name="psum", bufs=8 // v["copy_span"],
                         space="PSUM") as psum_pool,
        ):
            wt = wpool.tile([128, N_PAIR * 128], tio)
            (nc.scalar if v["w_on_scalar"] else nc.sync).dma_start(wt[:], w_d[:])

            x_r = x_d.rearrange("p k b -> k p b")
            y_r = y_d.rearrange("p k b -> k p b")

            def phased_body():
                # Pure-read phase (all x loads), then pure-write phase
                # (stores gated on the last load) -- avoids HBM read/write
                # bus turnaround at packet granularity.
                from concourse.tile_rust import add_dep_helper

                sc = v["store_chunks"]
                xts = []
                last_ld = None
                for p in range(N_PAIR):
                    xt = xpool.tile([128, 1, BS], tio)
                    last_ld = nc.sync.dma_start(xt[:, :, :], x_r[:, p:p + 1, :])
                    xts.append(xt)
                for p in range(N_PAIR):
                    yt = ypool.tile([128, 1, BS], tio)
                    for n in range(NT):
                        ps = psum_pool.tile([128, FREE], f32)
                        nc.tensor.matmul(
                            ps[:],
                            wt[:, p * 128:(p + 1) * 128],
                            xts[p][:, 0, n * FREE:(n + 1) * FREE],
                            start=True, stop=True,
                        )
                        if n >= v["copy_act_from"]:
                            nc.scalar.copy(yt[:, 0, n * FREE:(n + 1) * FREE],
                                           ps[:])
                        else:
                            nc.vector.tensor_copy(
                                yt[:, 0, n * FREE:(n + 1) * FREE], ps[:])
                    for i in range(sc):
                        w_ = BS // sc
                        st = nc.scalar.dma_start(
                            y_r[:, p:p + 1, i * w_:(i + 1) * w_],
                            yt[:, :, i * w_:(i + 1) * w_])
                        if p == 0 and i == 0:
                            add_dep_helper(
                                st.ins, last_ld.ins, sync=True,
                                reason="phase: stores begin after all loads")

            def body():
                lc, lm = v["load_chunks"], v["load_merge"]
                npg = N_PAIR // lm

                def engines_for(pg):
                    # (load_engine, store_engine) per pair-group. "halves":
                    # each ring carries half the loads and the other half's
                    # stores, grouped to minimize read<->write direction
                    # switches per ring.
                    if v["ring"] == "halves":
                        if pg < npg // 2:
                            return nc.sync, nc.scalar
                        return nc.scalar, nc.sync
                    if v["ring"] == "self_halves":
                        if pg < npg // 2:
                            return nc.sync, nc.sync
                        return nc.scalar, nc.scalar
                    if v["alt_engines"] and pg % 2:
                        return nc.scalar, nc.sync
                    return nc.sync, nc.scalar

                def lc_for(pg):
                    if pg == npg - 1 and v["last_lc"]:
                        return v["last_lc"]
                    return lc

                def do_load(pg):
                    ld_eng, _ = engines_for(pg)
                    my_lc = lc_for(pg)
                    xt = xpool.tile([128, lm, BS], tio)
                    for i in range(my_lc):
                        w_ = BS // my_lc
                        ld_eng.dma_start(
                            xt[:, :, i * w_:(i + 1) * w_],
                            x_r[:, pg * lm:(pg + 1) * lm, i * w_:(i + 1) * w_])
                    return xt

                xts = {}
                if v["phase_loads"]:
                    for pg in range(npg):
                        xts[pg] = do_load(pg)

                sm = v["store_merge"]
                yt = None
                for pg in range(npg):
                    sc = v["store_chunks"]
                    if pg == npg - 1 and v["last_sc"]:
                        sc = v["last_sc"]
                    xt = xts[pg] if v["phase_loads"] else do_load(pg)
                    if pg % sm == 0:
                        yt = ypool.tile([128, lm * sm, BS], tio)
                    span = v["copy_span"]
                    for j in range(lm):
                        p = pg * lm + j
                        jj = (pg % sm) * lm + j
                        for n2 in range(NT // span):
                            ps = psum_pool.tile([128, span * FREE], f32)
                            for s in range(span):
                                n = n2 * span + s
                                nc.tensor.matmul(
                                    ps[:, s * FREE:(s + 1) * FREE],
                                    wt[:, p * 128:(p + 1) * 128],
                                    xt[:, j, n * FREE:(n + 1) * FREE],
                                    start=True, stop=True,
                                )
                            lo = n2 * span * FREE
                            hi = lo + span * FREE
                            if n2 * span >= v["copy_act_from"]:
                                nc.scalar.copy(yt[:, jj, lo:hi], ps[:])
                            else:
                                nc.vector.tensor_copy(yt[:, jj, lo:hi], ps[:])
                    if pg % sm == sm - 1:
                        _, st_eng = engines_for(pg)
                        base_p = (pg - sm + 1) * lm
                        for i in range(sc * lm * sm):
                            w_ = BS // sc
                            j, ii = divmod(i, sc)
                            st_eng.dma_start(
                                y_r[:, base_p + j, ii * w_:(ii + 1) * w_],
                                yt[:, j, ii * w_:(ii + 1) * w_])

            def probe_body():
                # DMA-only bandwidth probes: no matmuls/copies, no deps
                # between loads and stores. probe in {loads, stores, both,
                # loads2, stores2, both2}; '2' = split across both rings.
                pr = v["probe"]
                do_ld = pr in ("loads", "both", "loads2", "both2", "both_h")
                do_st = pr in ("stores", "both", "stores2", "both2", "both_h")
                for p in range(N_PAIR):
                    if do_ld:
                        if pr in ("loads2", "both2"):
                            eng = nc.scalar if p % 2 else nc.sync
                        elif pr == "both_h":
                            eng = nc.sync if p < N_PAIR // 2 else nc.scalar
                        else:
                            eng = nc.sync
                        xt = xpool.tile([128, 1, BS], f32 if v["io"] == "f32"
                                        else mybir.dt.float16)
                        eng.dma_start(xt[:, :, :], x_r[:, p:p + 1, :])
                    if do_st:
                        if pr in ("stores2", "both2"):
                            eng = nc.sync if p % 2 == 0 else nc.scalar
                        elif pr == "both_h":
                            eng = nc.scalar if p < N_PAIR // 2 else nc.sync
                        else:
                            eng = nc.scalar
                        yt = ypool.tile([128, 1, BS], f32 if v["io"] == "f32"
                                        else mybir.dt.float16)
                        nc.vector.memset(yt[:, :, :1], 0.0)
                        eng.dma_start(y_r[:, p:p + 1, :], yt[:, :, :])

            the_body = (probe_body if v["probe"]
                        else phased_body if v["phased"] else body)
            u = v.get("unroll", 1)
            if repeat == 1:
                the_body()
            elif u == 1:
                with tc.For_i(0, repeat, 1):
                    the_body()
            else:
                n_loop = repeat // u
                with tc.For_i(0, n_loop, 1):
                    for _ in range(u):
                        the_body()
                for _ in range(repeat - n_loop * u):
                    the_body()

    nc.compile()
    return nc


def _get_program(repeat: int = 1, variant: dict | None = None):
    key = ("nc", repeat, tuple(sorted((variant or {}).items())))
    if key not in _CACHE:
        _CACHE[key] = _build_program(repeat, variant)
    return _CACHE[key]


IO_DTYPE = np.float16


def _pack_weights(W: np.ndarray) -> np.ndarray:
    """[16, 64, 64] -> [128, 8*128] lhsT layout: col p*128+m, row k holds
    block_diag(W[2p].T, W[2p+1].T)[k, m]."""
    WD = np.zeros((N_PAIR, 128, 128), dtype=np.float32)
    for p in range(N_PAIR):
        WD[p, :D_BLK, :D_BLK] = W[2 * p].T
        WD[p, D_BLK:, D_BLK:] = W[2 * p + 1].T
    return np.ascontiguousarray(
        WD.transpose(1, 0, 2).reshape(128, N_PAIR * 128).astype(IO_DTYPE))


def _get_runner():
    """Build (once) the jitted 8-core dispatch for the bass program.

    Mirrors concourse.bass2jax.run_bass_via_pjrt's multi-core branch, but is
    cached so repeat kernel() calls skip retracing, and takes pre-concatenated
    global inputs to avoid an extra host copy.
    """
    if "runner" in _CACHE:
        return _CACHE["runner"]

    import jax
    from concourse import mybir
    from concourse.bass2jax import (
        _bass_exec_p,
        install_neuronx_cc_hook,
        partition_id_tensor,
    )
    from jax.experimental.shard_map import shard_map
    from jax.sharding import Mesh, NamedSharding, PartitionSpec

    install_neuronx_cc_hook()
    nc = _get_program()

    partition_name = nc.partition_id_tensor.name if nc.partition_id_tensor else None
    in_names, out_names, out_avals, out_shapes = [], [], [], []
    for alloc in nc.m.functions[0].allocations:
        if not isinstance(alloc, mybir.MemoryLocationSet):
            continue
        name = alloc.memorylocations[0].name
        if alloc.kind == "ExternalInput":
            if name != partition_name:
                in_names.append(name)
        elif alloc.kind == "ExternalOutput":
            out_names.append(name)
            shape = tuple(alloc.tensor_shape)
            dtype = mybir.dt.np(alloc.dtype)
            out_avals.append(jax.core.ShapedArray(shape, dtype))
            out_shapes.append((shape, dtype))
    n_params = len(in_names)
    n_outs = len(out_avals)
    all_in_names = in_names + out_names
    if partition_name is not None:
        all_in_names.append(partition_name)
    donate = tuple(range(n_params, n_params + n_outs))

    def _body(*args):
        operands = list(args)
        if partition_name is not None:
            operands.append(partition_id_tensor())
        outs = _bass_exec_p.bind(
            *operands,
            out_avals=tuple(out_avals),
            in_names=tuple(all_in_names),
            out_names=tuple(out_names),
            lowering_input_output_aliases=(),
            sim_require_finite=True,
            sim_require_nnan=True,
            nc=nc,
        )
        return tuple(outs)

    devices = jax.devices()[:N_CORES]
    mesh = Mesh(np.asarray(devices), ("core",))
    in_specs = (PartitionSpec("core"),) * (n_params + n_outs)
    out_specs = (PartitionSpec("core"),) * n_outs
    sharded = jax.jit(
        shard_map(_body, mesh=mesh, in_specs=in_specs, out_specs=out_specs,
                  check_rep=False),
        donate_argnums=donate,
        keep_unused=True,
    )
    shard = NamedSharding(mesh, PartitionSpec("core"))

    # Donated output buffers: create on-device (no 128 MB host->device
    # transfer per call). Fall back to host staging if the backend rejects
    # the output-only jit.
    import jax.numpy as jnp

    zero_shapes = [((shape[0] * N_CORES,) + shape[1:], dtype)
                   for shape, dtype in out_shapes]
    zeros_jit = jax.jit(
        lambda: tuple(jnp.zeros(s, d) for s, d in zero_shapes),
        out_shardings=tuple(shard for _ in zero_shapes),
    )

    def host_zeros():
        return [jax.device_put(np.zeros(s, d), shard) for s, d in zero_shapes]

    try:
        jax.block_until_ready(zeros_jit())
        make_zeros = lambda: list(zeros_jit())  # noqa: E731
    except Exception:
        make_zeros = host_zeros

    def run(global_ins: dict):
        """global_ins: name -> concatenated [N_CORES*dim0, ...] array."""
        dev_in = [jax.device_put(global_ins[name], shard)
                  for name in in_names]
        outs = sharded(*dev_in, *make_zeros())
        return {name: np.asarray(o) for name, o in zip(out_names, outs)}

    _CACHE["runner"] = run
    return run


def _make_global_inputs(W: np.ndarray, inp: np.ndarray) -> dict:
    # Global sharded inputs (axis 0 split across cores by shard_map):
    #   w: [N_CORES*128, 1024] -- weights replicated per core
    #   x: [N_CORES*8, 128, BS] -- core c gets inp[:, c*BS:(c+1)*BS]
    w_host = _pack_weights(np.asarray(W, dtype=np.float32))
    w_global = np.tile(w_host, (N_CORES, 1))
    x_global = np.ascontiguousarray(
        np.asarray(inp, dtype=np.float32)
        .reshape(N_PAIR, 128, N_CORES, BS).transpose(2, 0, 1, 3)
        .astype(IO_DTYPE)
    ).reshape(N_CORES * N_PAIR, 128, BS)
    return {"w": w_global, "x": x_global}


def _kernel_direct(w_host: np.ndarray, inp: np.ndarray) -> np.ndarray:
    w_global = np.tile(w_host, (N_CORES, 1))
    x_global = np.ascontiguousarray(
        inp.reshape(N_PAIR, 128, N_CORES, BS).transpose(2, 0, 1, 3)
        .astype(IO_DTYPE)
    ).reshape(N_CORES * N_PAIR, 128, BS)

    run = _get_runner()
    outs = run({"w": w_global, "x": x_global})

    y = outs["y"].reshape(N_CORES, N_PAIR, 128, BS)
    return np.ascontiguousarray(
        y.transpose(1, 2, 0, 3).astype(np.float32)).reshape(D_TOTAL, B)


def _kernel_via_spmd(w_host: np.ndarray, inp: np.ndarray) -> np.ndarray:
    from concourse.bass_utils import run_bass_kernel_spmd

    nc = _get_program()
    in_maps = []
    for c in range(N_CORES):
        x_shard = np.ascontiguousarray(
            inp[:, c * BS:(c + 1) * BS].astype(IO_DTYPE))
        in_maps.append({"w": w_host, "x": x_shard.reshape(N_PAIR, 128, BS)})
    res = run_bass_kernel_spmd(nc, in_maps, core_ids=list(range(N_CORES)))
    out = np.empty((D_TOTAL, B), dtype=np.float32)
    for c in range(N_CORES):
        out[:, c * BS:(c + 1) * BS] = (
            res.results[c]["y"].reshape(D_TOTAL, BS).astype(np.float32))
    return out


def kernel(W: np.ndarray, inp: np.ndarray) -> np.ndarray:
    W = np.asarray(W, dtype=np.float32)
    inp = np.asarray(inp, dtype=np.float32)
    assert W.shape == (H, D_BLK, D_BLK) and inp.shape == (D_TOTAL, B)

    w_host = _pack_weights(W)

    try:
        from concourse._compat import axon_active
        use_direct = axon_active()
    except Exception:
        use_direct = False

    if use_direct:
        try:
            return _kernel_direct(w_host, inp)
        except Exception:
            # Transient device wedges (NRT_EXEC_UNIT_UNRECOVERABLE) have been
            # observed to need ~60 s to clear; retry once after a long
            # backoff, then fall back to the run_bass_kernel_spmd path.
            import time
            time.sleep(45)
            try:
                return _kernel_direct(w_host, inp)
            except Exception:
                time.sleep(30)
    return _kernel_via_spmd(w_host, inp)


if __name__ == "__main__":
    rng = np.random.default_rng(0)
    W = rng.standard_normal((H, D_BLK, D_BLK), dtype=np.float32)
    inp = rng.standard_normal((D_TOTAL, B), dtype=np.float32)
    out = kernel(W, inp)
    ref = np.einsum("hij,hjb->hib", W, inp.reshape(H, D_BLK, B)).reshape(D_TOTAL, B)
    err = np.abs(out - ref).max() / max(np.abs(ref).max(), 1e-9)
    print("self-check rel err:", err)
    assert err < 2e-2, f"FAIL rel={err}"



# revision 24
# speedup vs baseline: 2.0482x; 1.0222x over previous
"""Block-diagonal matmul (BlockLinear) on 8 Trainium2 NeuronCores.

Problem: W [16, 64, 64] f32 stacked square blocks; inp [1024, 32768] f32.
out = block_diag(W) @ inp, i.e. per-block out[h] = W[h] @ inp[h*64:(h+1)*64, :].

Strategy (data parallel over the batch axis, per the sharding hint):
  - Shard inp / out along B=32768 across 8 cores (4096 columns each).
  - Host-side, pack the 16 64x64 blocks into 8 block-diagonal 128x128 pairs,
    pre-transposed for the TensorE "lhsT" stationary operand (so the device
    does no transposes and the full 128-partition dim is used), and cast all
    HBM-facing data to fp16 (tolerance allows it: fp16 matmul with f32 PSUM
    accumulation keeps rel err ~1e-3, and it halves HBM traffic, which is
    the binding constraint).
  - Per core: phase-separated DMA across the two HWDGE rings (sync+scalar).
    Each iteration both rings first carry 4 of the 8 [128, 4096] fp16 pair
    slabs in (read-pure phase), then 4 whole-pair stores each (write-pure
    phase): per-core HBM bandwidth is shared between directions, so
    direction-pure phases with one read<->write switch per ring per
    iteration beat a loads-ring/stores-ring split by ~8%. Compute per pair:
    8 matmuls of N=512 into f32 PSUM banks, PSUM->SBUF copies with fused
    f32->fp16 conversion split DVE(3/4)/ACT(1/4).
  - Host upconverts the fp16 device output back to f32.

Memory-bound: 16 MiB HBM traffic per core per iteration; measured pure-
direction DMA rate is ~344 GB/s per core per direction (24.4 us for 8 MiB),
so the phase-additive floor is ~48.8 us. Measured (repeat-loop slope on
HW): ~49-50 us per core, vs ~101 us for the f32 version of the same
pipeline.
"""

import os
import sys

import numpy as np

for _p in ("/opt/trn_rl_repo", "/opt/pypackages"):
    if os.path.isdir(_p) and _p not in sys.path:
        sys.path.append(_p)

H, D_BLK = 16, 64
D_TOTAL = H * D_BLK            # 1024
B = 32768
N_CORES = 8
BS = B // N_CORES              # 4096 batch columns per core
N_PAIR = H // 2                # 8 pairs of blocks -> 128 partitions each
FREE = 512                     # one PSUM bank of f32
NT = BS // FREE                # 8 matmuls per pair

_CACHE = {}


def _build_program(repeat: int = 1, variant: dict | None = None):
    import concourse.bacc as bacc
    import concourse.tile as tile
    from concourse import mybir

    # Defaults = best HW-measured variant (A/B at same For_i repeat R):
    # deep double-buffering, stores in 2 chunks on the scalar HWDGE
    # ring (separate FIFO from loads), last pair stored in 4 finer chunks,
    # PSUM->SBUF copies in 2-bank [128,1024] chunks split DVE(3)/ACT(1) to
    # shorten the per-pair copy chain, weight load off the sync ring.
    # io=fp16: HBM-facing tensors in float16 (matmul in fp16, PSUM f32,
    # conversion fused into the PSUM->SBUF copies) -- halves HBM traffic.
    # Best HW-measured config: fp16 IO; per iteration both HWDGE rings
    # first carry half the loads each (read-pure phase), then half the
    # stores each (write-pure phase) -- one read<->write direction switch
    # per ring per iteration, which matters because per-core HBM bandwidth
    # is shared between directions and mixing costs ~8%. Single whole-pair
    # store DMAs, deep pools, 4x unrolled hardware loop.
    v = dict(bufs_x=8, bufs_y=6, store_chunks=1, load_chunks=1,
             alt_engines=False, copy_act_from=6, last_sc=None,
             w_on_scalar=True, load_merge=1, phased=False, copy_span=2,
             last_lc=None, io="fp16", probe=None, ring="halves",
             phase_loads=True, store_merge=1, unroll=4)
    v.update(variant or {})

    f32 = mybir.dt.float32
    tio = {"fp16": mybir.dt.float16, "bf16": mybir.dt.bfloat16,
           "f32": mybir.dt.float32}[v["io"]]
    nc = bacc.Bacc("TRN2", target_bir_lowering=False, debug=False,
                   num_devices=N_CORES)

    w_d = nc.dram_tensor("w", (128, N_PAIR * 128), tio, kind="ExternalInput")
    x_d = nc.dram_tensor("x", (N_PAIR, 128, BS), tio, kind="ExternalInput")
    y_d = nc.dram_tensor("y", (N_PAIR, 128, BS), tio, kind="ExternalOutput")

    with tile.TileContext(nc) as tc:
        with (
            tc.tile_pool(name="wpool", bufs=1) as wpool,
            tc.tile_pool(name="xpool", bufs=v["bufs_x"]) as xpool,
            tc.tile_pool(name="ypool", bufs=v["bufs_y"]) as ypool,
            tc.tile_pool(name="psum", bufs=8 // v["copy_span"],
                         space="PSUM") as psum_pool,
        ):
            wt = wpool.tile([128, N_PAIR * 128], tio)
            (nc.scalar if v["w_on_scalar"] else nc.sync).dma_start(wt[:], w_d[:])

            x_r = x_d.rearrange("p k b -> k p b")
            y_r = y_d.rearrange("p k b -> k p b")

            def phased_body():
                # Pure-read phase (all x loads), then pure-write phase
                # (stores gated on the last load) -- avoids HBM read/write
                # bus turnaround at packet granularity.
                from concourse.tile_rust import add_dep_helper

                sc = v["store_chunks"]
                xts = []
                last_ld = None
                for p in range(N_PAIR):
                    xt = xpool.tile([128, 1, BS], tio)
                    last_ld = nc.sync.dma_start(xt[:, :, :], x_r[:, p:p + 1, :])
                    xts.append(xt)
                for p in range(N_PAIR):
                    yt = ypool.tile([128, 1, BS], tio)
                    for n in range(NT):
                        ps = psum_pool.tile([128, FREE], f32)
                        nc.tensor.matmul(
                            ps[:],
                            wt[:, p * 128:(p + 1) * 128],
                            xts[p][:, 0, n * FREE:(n + 1) * FREE],
                            start=True, stop=True,
                        )
                        if n >= v["copy_act_from"]:
                            nc.scalar.copy(yt[:, 0, n * FREE:(n + 1) * FREE],
                                           ps[:])
                        else:
                            nc.vector.tensor_copy(
                                yt[:, 0, n * FREE:(n + 1) * FREE], ps[:])
                    for i in range(sc):
                        w_ = BS // sc
                        st = nc.scalar.dma_start(
                            y_r[:, p:p + 1, i * w_:(i + 1) * w_],
                            yt[:, :, i * w_:(i + 1) * w_])
                        if p == 0 and i == 0:
                            add_dep_helper(
                                st.ins, last_ld.ins, sync=True,
                                reason="phase: stores begin after all loads")

            def body():
                lc, lm = v["load_chunks"], v["load_merge"]
                npg = N_PAIR // lm

                def engines_for(pg):
                    # (load_engine, store_engine) per pair-group. "halves":
                    # each ring carries half the loads and the other half's
                    # stores, grouped to minimize read<->write direction
                    # switches per ring.
                    if v["ring"] == "halves":
                        if pg < npg // 2:
                            return nc.sync, nc.scalar
                        return nc.scalar, nc.sync
                    if v["ring"] == "self_halves":
                        if pg < npg // 2:
                            return nc.sync, nc.sync
                        return nc.scalar, nc.scalar
                    if v["alt_engines"] and pg % 2:
                        return nc.scalar, nc.sync
                    return nc.sync, nc.scalar

                def lc_for(pg):
                    if pg == npg - 1 and v["last_lc"]:
                        return v["last_lc"]
                    return lc

                def do_load(pg):
                    ld_eng, _ = engines_for(pg)
                    my_lc = lc_for(pg)
                    xt = xpool.tile([128, lm, BS], tio)
                    for i in range(my_lc):
                        w_ = BS // my_lc
                        ld_eng.dma_start(
                            xt[:, :, i * w_:(i + 1) * w_],
                            x_r[:, pg * lm:(pg + 1) * lm, i * w_:(i + 1) * w_])
                    return xt

                xts = {}
                if v["phase_loads"]:
                    for pg in range(npg):
                        xts[pg] = do_load(pg)

                sm = v["store_merge"]
                yt = None
                for pg in range(npg):
                    sc = v["store_chunks"]
                    if pg == npg - 1 and v["last_sc"]:
                        sc = v["last_sc"]
                    xt = xts[pg] if v["phase_loads"] else do_load(pg)
                    if pg % sm == 0:
                        yt = ypool.tile([128, lm * sm, BS], tio)
                    span = v["copy_span"]
                    for j in range(lm):
                        p = pg * lm + j
                        jj = (pg % sm) * lm + j
                        for n2 in range(NT // span):
                            ps = psum_pool.tile([128, span * FREE], f32)
                            for s in range(span):
                                n = n2 * span + s
                                nc.tensor.matmul(
                                    ps[:, s * FREE:(s + 1) * FREE],
                                    wt[:, p * 128:(p + 1) * 128],
                                    xt[:, j, n * FREE:(n + 1) * FREE],
                                    start=True, stop=True,
                                )
                            lo = n2 * span * FREE
                            hi = lo + span * FREE
                            if n2 * span >= v["copy_act_from"]:
                                nc.scalar.copy(yt[:, jj, lo:hi], ps[:])
                            else:
                                nc.vector.tensor_copy(yt[:, jj, lo:hi], ps[:])
                    if pg % sm == sm - 1:
                        _, st_eng = engines_for(pg)
                        base_p = (pg - sm + 1) * lm
                        for i in range(sc * lm * sm):
                            w_ = BS // sc
                            j, ii = divmod(i, sc)
                            st_eng.dma_start(
                                y_r[:, base_p + j, ii * w_:(ii + 1) * w_],
                                yt[:, j, ii * w_:(ii + 1) * w_])

            def probe_body():
                # DMA-only bandwidth probes: no matmuls/copies, no deps
                # between loads and stores. probe in {loads, stores, both,
                # loads2, stores2, both2}; '2' = split across both rings.
                pr = v["probe"]
                do_ld = pr in ("loads", "both", "loads2", "both2", "both_h")
                do_st = pr in ("stores", "both", "stores2", "both2", "both_h")
                for p in range(N_PAIR):
                    if do_ld:
                        if pr in ("loads2", "both2"):
                            eng = nc.scalar if p % 2 else nc.sync
                        elif pr == "both_h":
                            eng = nc.sync if p < N_PAIR // 2 else nc.scalar
                        else:
                            eng = nc.sync
                        xt = xpool.tile([128, 1, BS], f32 if v["io"] == "f32"
                                        else mybir.dt.float16)
                        eng.dma_start(xt[:, :, :], x_r[:, p:p + 1, :])
                    if do_st:
                        if pr in ("stores2", "both2"):
                            eng = nc.sync if p % 2 == 0 else nc.scalar
                        elif pr == "both_h":
                            eng = nc.scalar if p < N_PAIR // 2 else nc.sync
                        else:
                            eng = nc.scalar
                        yt = ypool.tile([128, 1, BS], f32 if v["io"] == "f32"
                                        else mybir.dt.float16)
                        nc.vector.memset(yt[:, :, :1], 0.0)
                        eng.dma_start(y_r[:, p:p + 1, :], yt[:, :, :])

            the_body = (probe_body if v["probe"]
                        else phased_body if v["phased"] else body)
            u = v.get("unroll", 1)
            if repeat == 1:
                the_body()
            elif u == 1:
                with tc.For_i(0, repeat, 1):
                    the_body()
            else:
                n_loop = repeat // u
                with tc.For_i(0, n_loop, 1):
                    for _ in range(u):
                        the_body()
                for _ in range(repeat - n_loop * u):
                    the_body()

    nc.compile()
    return nc


def _get_program(repeat: int = 1, variant: dict | None = None):
    key = ("nc", repeat, tuple(sorted((variant or {}).items())))
    if key not in _CACHE:
        _CACHE[key] = _build_program(repeat, variant)
    return _CACHE[key]


IO_DTYPE = np.float16


def _pack_weights(W: np.ndarray) -> np.ndarray:
    """[16, 64, 64] -> [128, 8*128] lhsT layout: col p*128+m, row k holds
    block_diag(W[2p].T, W[2p+1].T)[k, m]."""
    WD = np.zeros((N_PAIR, 128, 128), dtype=np.float32)
    for p in range(N_PAIR):
        WD[p, :D_BLK, :D_BLK] = W[2 * p].T
        WD[p, D_BLK:, D_BLK:] = W[2 * p + 1].T
    return np.ascontiguousarray(
        WD.transpose(1, 0, 2).reshape(128, N_PAIR * 128).astype(IO_DTYPE))


def _get_runner():
    """Build (once) the jitted 8-core dispatch for the bass program.

    Mirrors concourse.bass2jax.run_bass_via_pjrt's multi-core branch, but is
    cached so repeat kernel() calls skip retracing, and takes pre-concatenated
    global inputs to avoid an extra host copy.
    """
    if "runner" in _CACHE:
        return _CACHE["runner"]

    import jax
    from concourse import mybir
    from concourse.bass2jax import (
        _bass_exec_p,
        install_neuronx_cc_hook,
        partition_id_tensor,
    )
    from jax.experimental.shard_map import shard_map
    from jax.sharding import Mesh, NamedSharding, PartitionSpec

    install_neuronx_cc_hook()
    nc = _get_program()

    partition_name = nc.partition_id_tensor.name if nc.partition_id_tensor else None
    in_names, out_names, out_avals, out_shapes = [], [], [], []
    for alloc in nc.m.functions[0].allocations:
        if not isinstance(alloc, mybir.MemoryLocationSet):
            continue
        name = alloc.memorylocations[0].name
        if alloc.kind == "ExternalInput":
            if name != partition_name:
                in_names.append(name)
        elif alloc.kind == "ExternalOutput":
            out_names.append(name)
            shape = tuple(alloc.tensor_shape)
            dtype = mybir.dt.np(alloc.dtype)
            out_avals.append(jax.core.ShapedArray(shape, dtype))
            out_shapes.append((shape, dtype))
    n_params = len(in_names)
    n_outs = len(out_avals)
    all_in_names = in_names + out_names
    if partition_name is not None:
        all_in_names.append(partition_name)
    donate = tuple(range(n_params, n_params + n_outs))

    def _body(*args):
        operands = list(args)
        if partition_name is not None:
            operands.append(partition_id_tensor())
        outs = _bass_exec_p.bind(
            *operands,
            out_avals=tuple(out_avals),
            in_names=tuple(all_in_names),
            out_names=tuple(out_names),
            lowering_input_output_aliases=(),
            sim_require_finite=True,
            sim_require_nnan=True,
            nc=nc,
        )
        return tuple(outs)

    devices = jax.devices()[:N_CORES]
    mesh = Mesh(np.asarray(devices), ("core",))
    in_specs = (PartitionSpec("core"),) * (n_params + n_outs)
    out_specs = (PartitionSpec("core"),) * n_outs
    sharded = jax.jit(
        shard_map(_body, mesh=mesh, in_specs=in_specs, out_specs=out_specs,
                  check_rep=False),
        donate_argnums=donate,
        keep_unused=True,
    )
    shard = NamedSharding(mesh, PartitionSpec("core"))

    # Donated output buffers: create on-device (no 128 MB host->device
    # transfer per call). Fall back to host staging if the backend rejects
    # the output-only jit.
    import jax.numpy as jnp

    zero_shapes = [((shape[0] * N_CORES,) + shape[1:], dtype)
                   for shape, dtype in out_shapes]
    zeros_jit = jax.jit(
        lambda: tuple(jnp.zeros(s, d) for s, d in zero_shapes),
        out_shardings=tuple(shard for _ in zero_shapes),
    )

    def host_zeros():
        return [jax.device_put(np.zeros(s, d), shard) for s, d in zero_shapes]

    try:
        jax.block_until_ready(zeros_jit())
        make_zeros = lambda: list(zeros_jit())  # noqa: E731
    except Exception:
        make_zeros = host_zeros

    def run(global_ins: dict):
        """global_ins: name -> concatenated [N_CORES*dim0, ...] array."""
        dev_in = [jax.device_put(global_ins[name], shard)
                  for name in in_names]
        outs = sharded(*dev_in, *make_zeros())
        return {name: np.asarray(o) for name, o in zip(out_names, outs)}

    _CACHE["runner"] = run
    return run


def _make_global_inputs(W: np.ndarray, inp: np.ndarray) -> dict:
    # Global sharded inputs (axis 0 split across cores by shard_map):
    #   w: [N_CORES*128, 1024] -- weights replicated per core
    #   x: [N_CORES*8, 128, BS] -- core c gets inp[:, c*BS:(c+1)*BS]
    w_host = _pack_weights(np.asarray(W, dtype=np.float32))
    w_global = np.tile(w_host, (N_CORES, 1))
    x_global = np.ascontiguousarray(
        np.asarray(inp, dtype=np.float32)
        .reshape(N_PAIR, 128, N_CORES, BS).transpose(2, 0, 1, 3)
        .astype(IO_DTYPE)
    ).reshape(N_CORES * N_PAIR, 128, BS)
    return {"w": w_global, "x": x_global}


def _kernel_direct(w_host: np.ndarray, inp: np.ndarray) -> np.ndarray:
    w_global = np.tile(w_host, (N_CORES, 1))
    x_global = np.ascontiguousarray(
        inp.reshape(N_PAIR, 128, N_CORES, BS).transpose(2, 0, 1, 3)
        .astype(IO_DTYPE)
    ).reshape(N_CORES * N_PAIR, 128, BS)

    run = _get_runner()
    outs = run({"w": w_global, "x": x_global})

    y = outs["y"].reshape(N_CORES, N_PAIR, 128, BS)
    return np.ascontiguousarray(
        y.transpose(1, 2, 0, 3).astype(np.float32)).reshape(D_TOTAL, B)


def _kernel_via_spmd(w_host: np.ndarray, inp: np.ndarray) -> np.ndarray:
    from concourse.bass_utils import run_bass_kernel_spmd

    nc = _get_program()
    in_maps = []
    for c in range(N_CORES):
        x_shard = np.ascontiguousarray(
            inp[:, c * BS:(c + 1) * BS].astype(IO_DTYPE))
        in_maps.append({"w": w_host, "x": x_shard.reshape(N_PAIR, 128, BS)})
    res = run_bass_kernel_spmd(nc, in_maps, core_ids=list(range(N_CORES)))
    out = np.empty((D_TOTAL, B), dtype=np.float32)
    for c in range(N_CORES):
        out[:, c * BS:(c + 1) * BS] = (
            res.results[c]["y"].reshape(D_TOTAL, BS).astype(np.float32))
    return out


def kernel(W: np.ndarray, inp: np.ndarray) -> np.ndarray:
    W = np.asarray(W, dtype=np.float32)
    inp = np.asarray(inp, dtype=np.float32)
    assert W.shape == (H, D_BLK, D_BLK) and inp.shape == (D_TOTAL, B)

    w_host = _pack_weights(W)

    try:
        from concourse._compat import axon_active
        use_direct = axon_active()
    except Exception:
        use_direct = False

    if use_direct:
        try:
            return _kernel_direct(w_host, inp)
        except Exception:
            # Transient device wedges (NRT_EXEC_UNIT_UNRECOVERABLE) have been
            # observed to need ~60 s to clear; retry once after a long
            # backoff, then fall back to the run_bass_kernel_spmd path.
            import time
            time.sleep(45)
            try:
                return _kernel_direct(w_host, inp)
            except Exception:
                time.sleep(30)
    return _kernel_via_spmd(w_host, inp)


if __name__ == "__main__":
    rng = np.random.default_rng(0)
    W = rng.standard_normal((H, D_BLK, D_BLK), dtype=np.float32)
    inp = rng.standard_normal((D_TOTAL, B), dtype=np.float32)
    out = kernel(W, inp)
    ref = np.einsum("hij,hjb->hib", W, inp.reshape(H, D_BLK, B)).reshape(D_TOTAL, B)
    err = np.abs(out - ref).max() / max(np.abs(ref).max(), 1e-9)
    print("self-check rel err:", err)
    assert err < 2e-2, f"FAIL rel={err}"

